# revision 25
# baseline (speedup 1.0000x reference)
"""Boid policy kernel for Trainium2 (8 NeuronCores).

Strategy
--------
Host: sort boids into 8 spatial patches (4x2 grid of 0.25 x 0.5 cells).
Core c owns the queries of patch c. All coordinates handed to core c are
shifted (mod 1) so patch c is centered at (0.5, 0.5); positions produced by
jax.random.uniform are multiples of 2^-23, so the shift is bit-exact when
done in integer lattice space. With the patch centered, every candidate pair
within unwrapped distance 0.5 has its unwrapped diff equal to the exact
toroidal diff, and every pair beyond is outside both interaction radii
either way -> no per-pair wrap handling on device.

Candidates are additionally sorted by (x-column of 0.125, then y) so each
128-candidate block is a tight spatial cluster. The host culls blocks per
core: blocks beyond perception reach of the patch rectangle are dropped
entirely; blocks beyond separation reach skip the separation mask/matmuls.
Sep-active blocks are ordered first so the device program is static.

Device (per core, j = candidate on partitions, i = query on free axis):
  dx2 = ACT Square(qx_bcast + (-cx_j))      (per-partition bias)
  dy2 = ACT Square(qy_bcast + (-cy_j))
  d2  = GPSIMD tensor_tensor add            (exact fp32, frees the DVE)
  perc = DVE (d2 <= 0.2^2) [f32r out], sep likewise on sep-active blocks
  PE: masked sums via matmuls with the tiny jdata matrix as stationary
      operand and the mask tile as moving operand (float32r), accumulated
      over the active blocks in PSUM:
        perc-sums: [count, vxhi, vxlo, vyhi, vylo, pxhi, pxlo, pyhi, pylo]
        sep-sums:  [count, pxhi, pxlo, pyhi, pylo]
      (hi/lo weight splits survive the f32r tfloat32 rounding exactly)

Host epilogue (f64): recover sum(mask*diff) = sum(mask*pc) - qc*count
(self-pair cancels), subtract self from count/velocity sums, normalize the
three steers, combine with weights, add noise, clip by norm.
"""

import numpy as np

import concourse.bass as bass
import concourse.bacc as bacc
import concourse.mybir as mybir
from concourse.tile import TileContext
from concourse.bass_utils import run_bass_kernel_spmd

N = 8192
NCORES = 8
NBLK = N // 128  # 64 candidate blocks
PERC2 = float(np.float32(0.2**2))
SEP2 = float(np.float32(0.02**2))
EPS = 1e-8
HX, HY = 0.125, 0.25  # patch half-extents
RCULL_P = 0.2 + 1e-3
RCULL_S = 0.02 + 1e-3

_CACHE = {}


def _build(C: int, njp: int, njs: int) -> bass.Bass:
    f32 = mybir.dt.float32
    f32r = mybir.dt.float32r
    AF = mybir.ActivationFunctionType
    ALU = mybir.AluOpType

    nc = bacc.Bacc()
    qxb_h = nc.declare_dram_parameter("qxb", [128, C], f32, isOutput=False)
    qyb_h = nc.declare_dram_parameter("qyb", [128, C], f32, isOutput=False)
    # jd cols: [0]=-cx [1]=-cy [2:11]=perc weights [1,vxhi,vxlo,vyhi,vylo,
    # pxhi,pxlo,pyhi,pylo] [11:16]=sep weights [1,pxhi,pxlo,pyhi,pylo]
    jd_h = nc.declare_dram_parameter("jd", [njp, 128, 16], f32, isOutput=False)
    outp_h = nc.declare_dram_parameter("outp", [9, C], f32, isOutput=True)
    outs_h = nc.declare_dram_parameter("outs", [5, C], f32, isOutput=True)

    chunks = [(s, min(s + 512, C)) for s in range(0, C, 512)]
    CSPL = (int(C * 0.59) + 63) & ~63  # DVE share of the d2 add; rest on GPSIMD

    with TileContext(nc) as tc:
        with (
            tc.tile_pool(name="const", bufs=1) as cpool,
            tc.tile_pool(name="jin", bufs=4) as jpool,
            tc.tile_pool(name="work", bufs=4) as wpool,
            tc.tile_pool(name="outb", bufs=1) as opool,
            tc.tile_pool(name="acc", bufs=1, space="PSUM") as apool,
        ):
            qx = cpool.tile([128, C], f32)
            nc.sync.dma_start(out=qx[:], in_=qxb_h[:, :])
            qy = cpool.tile([128, C], f32)
            nc.sync.dma_start(out=qy[:], in_=qyb_h[:, :])

            accp = apool.tile([9, C], f32)
            accs = apool.tile([5, C], f32)

            for b in range(njp):
                jt = jpool.tile([128, 16], f32)
                nc.sync.dma_start(out=jt[:], in_=jd_h[b])
                # weights copy, rounded to f32r on write (hi cols exact,
                # lo cols' rounding bounded by 2^-21)
                jw = jpool.tile([128, 14], f32r, tag="jw")
                nc.gpsimd.tensor_copy(out=jw[:], in_=jt[:, 2:16])

                dx2 = wpool.tile([128, C], f32, tag="dx2")
                nc.scalar.activation(
                    out=dx2[:], in_=qx[:], func=AF.Square, bias=jt[:, 0:1], scale=1.0
                )
                dy2 = wpool.tile([128, C], f32, tag="dy2")
                nc.scalar.activation(
                    out=dy2[:], in_=qy[:], func=AF.Square, bias=jt[:, 1:2], scale=1.0
                )
                # d2 add split by column range: DVE and GPSIMD in parallel
                d2 = wpool.tile([128, C], f32, tag="d2")
                nc.vector.tensor_tensor(
                    out=d2[:, 0:CSPL], in0=dx2[:, 0:CSPL], in1=dy2[:, 0:CSPL],
                    op=ALU.add
                )
                nc.gpsimd.tensor_tensor(
                    out=d2[:, CSPL:C], in0=dx2[:, CSPL:C], in1=dy2[:, CSPL:C],
                    op=ALU.add
                )
                pm = wpool.tile([128, C], f32r, tag="pm")
                nc.vector.tensor_scalar(
                    out=pm[:], in0=d2[:], scalar1=PERC2, scalar2=None, op0=ALU.is_le
                )
                for a, e in chunks:
                    nc.tensor.matmul(
                        out=accp[:, a:e],
                        lhsT=jw[:, 0:9],
                        rhs=pm[:, a:e],
                        start=(b == 0),
                        stop=(b == njp - 1),
                    )
                if b < njs:
                    sm = wpool.tile([128, C], f32r, tag="sm")
                    nc.gpsimd.tensor_scalar(
                        out=sm[:], in0=d2[:], scalar1=SEP2, scalar2=None,
                        op0=ALU.is_le
                    )
                    for a, e in chunks:
                        nc.tensor.matmul(
                            out=accs[:, a:e],
                            lhsT=jw[:, 9:14],
                            rhs=sm[:, a:e],
                            start=(b == 0),
                            stop=(b == njs - 1),
                        )

            po = opool.tile([9, C], f32)
            nc.scalar.copy(out=po[:], in_=accp[:])
            so = opool.tile([5, C], f32)
            nc.scalar.copy(out=so[:], in_=accs[:])
            nc.sync.dma_start(out=outp_h[:, :], in_=po[:])
            nc.sync.dma_start(out=outs_h[:, :], in_=so[:])
    nc.finalize()
    return nc


def _get_nc(C: int, njp: int, njs: int) -> bass.Bass:
    key = (C, njp, njs)
    if key not in _CACHE:
        _CACHE[key] = _build(C, njp, njs)
    return _CACHE[key]


def _hilo(v64):
    hi = np.round(v64 * 1024.0) / 1024.0
    lo = (v64 - hi).astype(np.float32)
    return hi.astype(np.float32), lo


def _prepare(pos, vel):
    """Host-side prep: patch sort, lattice-exact shifts, block culling,
    packed device inputs.

    Returns (in_maps, sels, qxcs, qycs, C, njp, njs).
    """
    n = pos.shape[0]
    assert n == N, f"expected {N} boids, got {n}"

    # --- patch assignment: 4 columns in x (0.25), 2 rows in y (0.5) ---
    ix = np.clip((pos[:, 0] * 4.0).astype(np.int64), 0, 3)
    iy = np.clip((pos[:, 1] * 2.0).astype(np.int64), 0, 1)
    patch = ix * 2 + iy
    counts = np.bincount(patch, minlength=NCORES)
    C = int(np.ceil(counts.max() / 64.0) * 64)
    C = max(C, 512)

    # --- candidate order: x-column (0.125) major, y minor -> tight blocks ---
    jcol = np.clip((pos[:, 0] * 8.0).astype(np.int64), 0, 7)
    order = np.lexsort((pos[:, 1], jcol))

    # --- lattice-exact coordinate shifts ---
    SCL = 1 << 23
    p64x = pos[:, 0].astype(np.float64)
    p64y = pos[:, 1].astype(np.float64)
    kx = np.round(p64x * SCL).astype(np.int64)
    ky = np.round(p64y * SCL).astype(np.int64)
    lattice = bool(
        np.all(kx.astype(np.float64) == p64x * SCL)
        and np.all(ky.astype(np.float64) == p64y * SCL)
        and kx.min() >= 0 and kx.max() < SCL
        and ky.min() >= 0 and ky.max() < SCL
    )

    vx64 = vel[:, 0].astype(np.float64)
    vy64 = vel[:, 1].astype(np.float64)

    percore = []
    for c in range(NCORES):
        sel = np.nonzero(patch == c)[0]
        axk = int(((c // 2) * 0.25 + 0.125) * SCL)
        ayk = int(((c % 2) * 0.5 + 0.25) * SCL)
        if lattice:
            sxk = (kx - axk + (SCL >> 1)) % SCL
            syk = (ky - ayk + (SCL >> 1)) % SCL
            cx = (sxk.astype(np.float64) / SCL).astype(np.float32)
            cy = (syk.astype(np.float64) / SCL).astype(np.float32)
        else:  # fallback: tiny (~1e-9) inexactness vs reference wrap
            cx = np.mod(p64x - axk / SCL + 0.5, 1.0).astype(np.float32)
            cy = np.mod(p64y - ayk / SCL + 0.5, 1.0).astype(np.float32)

        # distance from each (shifted, sorted) candidate to the patch rect
        scx = cx[order].astype(np.float64)
        scy = cy[order].astype(np.float64)
        ddx = np.maximum(np.abs(scx - 0.5) - HX, 0.0)
        ddy = np.maximum(np.abs(scy - 0.5) - HY, 0.0)
        dd2 = ddx * ddx + ddy * ddy
        bd2 = dd2.reshape(NBLK, 128).min(axis=1)
        bsep = np.nonzero(bd2 <= RCULL_S * RCULL_S)[0]
        bperc = np.nonzero((bd2 <= RCULL_P * RCULL_P) & (bd2 > RCULL_S * RCULL_S))[0]
        percore.append((sel, cx, cy, scx, scy, bsep, bperc))

    njs = max(len(pc[5]) for pc in percore)
    njp = njs + max(len(pc[6]) for pc in percore)

    in_maps = []
    sels = []
    qxcs = []
    qycs = []
    for c in range(NCORES):
        sel, cx, cy, scx, scy, bsep, bperc = percore[c]
        nq = len(sel)
        qxp = np.full(C, 0.5, np.float32)
        qxp[:nq] = cx[sel]
        qyp = np.full(C, 0.5, np.float32)
        qyp[:nq] = cy[sel]
        qxb = np.ascontiguousarray(np.broadcast_to(qxp, (128, C)))
        qyb = np.ascontiguousarray(np.broadcast_to(qyp, (128, C)))

        # block list: sep-active first (padded to njs), then perc-only
        blocks = list(bsep) + [-1] * (njs - len(bsep)) + list(bperc)
        blocks += [-1] * (njp - len(blocks))

        scx32 = scx.astype(np.float32)
        scy32 = scy.astype(np.float32)
        svx = vx64[order]
        svy = vy64[order]
        jd = np.empty((njp, 128, 16), np.float32)
        for bi, b in enumerate(blocks):
            if b < 0:  # sentinel: far away, zero contributions
                jd[bi] = 0.0
                jd[bi, :, 0] = -50.0
                jd[bi, :, 1] = -50.0
                continue
            s = slice(128 * b, 128 * (b + 1))
            pxhi, pxlo = _hilo(scx[s] - 0.5)
            pyhi, pylo = _hilo(scy[s] - 0.5)
            vxhi, vxlo = _hilo(svx[s])
            vyhi, vylo = _hilo(svy[s])
            jd[bi, :, 0] = -scx32[s]
            jd[bi, :, 1] = -scy32[s]
            jd[bi, :, 2] = 1.0
            jd[bi, :, 3] = vxhi
            jd[bi, :, 4] = vxlo
            jd[bi, :, 5] = vyhi
            jd[bi, :, 6] = vylo
            jd[bi, :, 7] = pxhi
            jd[bi, :, 8] = pxlo
            jd[bi, :, 9] = pyhi
            jd[bi, :, 10] = pylo
            jd[bi, :, 11] = 1.0
            jd[bi, :, 12] = pxhi
            jd[bi, :, 13] = pxlo
            jd[bi, :, 14] = pyhi
            jd[bi, :, 15] = pylo

        in_maps.append({"qxb": qxb, "qyb": qyb, "jd": jd})
        sels.append(sel)
        qxcs.append(qxp[:nq].astype(np.float64) - 0.5)
        qycs.append(qyp[:nq].astype(np.float64) - 0.5)
    return in_maps, sels, qxcs, qycs, C, njp, njs


def kernel(position, velocity, noise, separation_weight, alignment_weight,
           cohesion_weight, noise_scale):
    pos = np.asarray(position, dtype=np.float32)
    vel = np.asarray(velocity, dtype=np.float32)
    noi = np.asarray(noise, dtype=np.float32)
    ws = float(separation_weight)
    wa = float(alignment_weight)
    wc = float(cohesion_weight)
    nsc = float(noise_scale)

    in_maps, sels, qxcs, qycs, C, njp, njs = _prepare(pos, vel)
    vx64 = vel[:, 0].astype(np.float64)
    vy64 = vel[:, 1].astype(np.float64)

    nc = _get_nc(C, njp, njs)
    res = run_bass_kernel_spmd(nc, in_maps, list(range(NCORES))).results

    # --- host epilogue (f64) ---
    out = np.zeros((N, 2), np.float32)
    for c in range(NCORES):
        sel = sels[c]
        nq = len(sel)
        P = res[c]["outp"].astype(np.float64)
        S = res[c]["outs"].astype(np.float64)
        cnt_all = P[0, :nq]
        svx, svy = P[1, :nq] + P[2, :nq], P[3, :nq] + P[4, :nq]
        spx, spy = P[5, :nq] + P[6, :nq], P[7, :nq] + P[8, :nq]
        scn = S[0, :nq]
        ssx, ssy = S[1, :nq] + S[2, :nq], S[3, :nq] + S[4, :nq]
        qxc, qyc = qxcs[c], qycs[c]

        cnt = cnt_all - 1.0
        # alignment: mean neighbor velocity minus own velocity
        vax = (svx - vx64[sel]) / cnt
        vay = (svy - vy64[sel]) / cnt
        dvx = vax - vx64[sel]
        dvy = vay - vy64[sel]
        # cohesion: mean toroidal diff (self term cancels exactly)
        pax = (spx - qxc * cnt_all) / cnt
        pay = (spy - qyc * cnt_all) / cnt
        # separation: -sum of masked diffs
        sepx = -(ssx - qxc * scn)
        sepy = -(ssy - qyc * scn)

        n1 = np.maximum(np.sqrt(sepx * sepx + sepy * sepy), EPS)
        n2 = np.maximum(np.sqrt(dvx * dvx + dvy * dvy), EPS)
        n3 = np.maximum(np.sqrt(pax * pax + pay * pay), EPS)

        ax = ws * sepx / n1 + wa * dvx / n2 + wc * pax / n3
        ay = ws * sepy / n1 + wa * dvy / n2 + wc * pay / n3
        ax = ax + nsc * noi[sel, 0].astype(np.float64)
        ay = ay + nsc * noi[sel, 1].astype(np.float64)
        nn = np.sqrt(ax * ax + ay * ay)
        f = np.where(nn > 1.0, 1.0 / np.maximum(nn, EPS), 1.0)
        out[sel, 0] = (ax * f).astype(np.float32)
        out[sel, 1] = (ay * f).astype(np.float32)
    return out


def run_with_trace(np_inputs):
    """Debug helper for test.py: run the device program with trace=True and
    return (exec_time_ns, profile_json_path_or_None)."""
    pos = np.asarray(np_inputs["position"], dtype=np.float32)
    vel = np.asarray(np_inputs["velocity"], dtype=np.float32)
    in_maps, _, _, _, C, njp, njs = _prepare(pos, vel)
    nc = _get_nc(C, njp, njs)
    r = run_bass_kernel_spmd(nc, in_maps, list(range(NCORES)), trace=True)
    return getattr(r, "exec_time_ns", None), getattr(r, "profile_json", None)


# revision 26
# speedup vs baseline: 3.3028x; 3.3028x over previous
"""Boid policy kernel for Trainium2 (8 NeuronCores).

Strategy
--------
Host: sort boids into 8 spatial patches (4x2 grid of 0.25 x 0.5 cells).
Core c owns the queries of patch c. All coordinates handed to core c are
shifted (mod 1) so patch c is centered at (0.5, 0.5); positions produced by
jax.random.uniform are multiples of 2^-23, so the shift is bit-exact when
done in integer lattice space. With the patch centered, every candidate pair
within unwrapped distance 0.5 has its unwrapped diff equal to the exact
toroidal diff, and every pair beyond is outside both interaction radii
either way -> no per-pair wrap handling on device.

Candidates are additionally sorted by (x-column of 0.125, then y) so each
128-candidate block is a tight spatial cluster. The host culls blocks per
core: blocks beyond perception reach of the patch rectangle are dropped
entirely; blocks beyond separation reach skip the separation mask/matmuls.
Sep-active blocks are ordered first so the device program is static.

Device (per core, j = candidate on partitions, i = query on free axis):
  dx2 = ACT Square(qx_bcast + (-cx_j))      (per-partition bias)
  dy2 = ACT Square(qy_bcast + (-cy_j))
  d2  = GPSIMD tensor_tensor add            (exact fp32, frees the DVE)
  perc = DVE (d2 <= 0.2^2) [f32r out], sep likewise on sep-active blocks
  PE: masked sums via matmuls with the tiny jdata matrix as stationary
      operand and the mask tile as moving operand (float32r), accumulated
      over the active blocks in PSUM:
        perc-sums: [count, vxhi, vxlo, vyhi, vylo, pxhi, pxlo, pyhi, pylo]
        sep-sums:  [count, pxhi, pxlo, pyhi, pylo]
      (hi/lo weight splits survive the f32r tfloat32 rounding exactly)

Host epilogue (f64): recover sum(mask*diff) = sum(mask*pc) - qc*count
(self-pair cancels), subtract self from count/velocity sums, normalize the
three steers, combine with weights, add noise, clip by norm.
"""

import numpy as np

import concourse.bass as bass
import concourse.bacc as bacc
import concourse.mybir as mybir
from concourse.tile import TileContext
from concourse.bass_utils import run_bass_kernel_spmd

N = 8192
NCORES = 8
NBLK = N // 128  # 64 candidate blocks
PERC2 = float(np.float32(0.2**2))
SEP2 = float(np.float32(0.02**2))
EPS = 1e-8
HX, HY = 0.125, 0.25  # patch half-extents
RCULL_P = 0.2 + 1e-3
RCULL_S = 0.02 + 1e-3

_CACHE = {}


def _build(C: int, njp: int, njs: int) -> bass.Bass:
    f32 = mybir.dt.float32
    f32r = mybir.dt.float32r
    AF = mybir.ActivationFunctionType
    ALU = mybir.AluOpType

    nc = bacc.Bacc()
    qxb_h = nc.declare_dram_parameter("qxb", [128, C], f32, isOutput=False)
    qyb_h = nc.declare_dram_parameter("qyb", [128, C], f32, isOutput=False)
    # jd cols: [0]=-cx [1]=-cy [2:11]=perc weights [1,vxhi,vxlo,vyhi,vylo,
    # pxhi,pxlo,pyhi,pylo] [11:16]=sep weights [1,pxhi,pxlo,pyhi,pylo]
    jd_h = nc.declare_dram_parameter("jd", [njp, 128, 16], f32, isOutput=False)
    outp_h = nc.declare_dram_parameter("outp", [9, C], f32, isOutput=True)
    outs_h = nc.declare_dram_parameter("outs", [5, C], f32, isOutput=True)

    chunks = [(s, min(s + 512, C)) for s in range(0, C, 512)]
    CSPL = (int(C * 0.59) + 63) & ~63  # DVE share of the d2 add; rest on GPSIMD

    with TileContext(nc) as tc:
        with (
            tc.tile_pool(name="const", bufs=1) as cpool,
            tc.tile_pool(name="jin", bufs=4) as jpool,
            tc.tile_pool(name="work", bufs=4) as wpool,
            tc.tile_pool(name="outb", bufs=1) as opool,
            tc.tile_pool(name="acc", bufs=1, space="PSUM") as apool,
        ):
            qx = cpool.tile([128, C], f32)
            nc.sync.dma_start(out=qx[:], in_=qxb_h[:, :])
            qy = cpool.tile([128, C], f32)
            nc.sync.dma_start(out=qy[:], in_=qyb_h[:, :])

            accp = apool.tile([9, C], f32)
            accs = apool.tile([5, C], f32)

            for b in range(njp):
                jt = jpool.tile([128, 16], f32)
                nc.sync.dma_start(out=jt[:], in_=jd_h[b])
                # weights copy, rounded to f32r on write (hi cols exact,
                # lo cols' rounding bounded by 2^-21)
                jw = jpool.tile([128, 14], f32r, tag="jw")
                nc.gpsimd.tensor_copy(out=jw[:], in_=jt[:, 2:16])

                dx2 = wpool.tile([128, C], f32, tag="dx2")
                nc.scalar.activation(
                    out=dx2[:], in_=qx[:], func=AF.Square, bias=jt[:, 0:1], scale=1.0
                )
                dy2 = wpool.tile([128, C], f32, tag="dy2")
                nc.scalar.activation(
                    out=dy2[:], in_=qy[:], func=AF.Square, bias=jt[:, 1:2], scale=1.0
                )
                # d2 add split by column range: DVE and GPSIMD in parallel
                d2 = wpool.tile([128, C], f32, tag="d2")
                nc.vector.tensor_tensor(
                    out=d2[:, 0:CSPL], in0=dx2[:, 0:CSPL], in1=dy2[:, 0:CSPL],
                    op=ALU.add
                )
                nc.gpsimd.tensor_tensor(
                    out=d2[:, CSPL:C], in0=dx2[:, CSPL:C], in1=dy2[:, CSPL:C],
                    op=ALU.add
                )
                pm = wpool.tile([128, C], f32r, tag="pm")
                nc.vector.tensor_scalar(
                    out=pm[:], in0=d2[:], scalar1=PERC2, scalar2=None, op0=ALU.is_le
                )
                for a, e in chunks:
                    nc.tensor.matmul(
                        out=accp[:, a:e],
                        lhsT=jw[:, 0:9],
                        rhs=pm[:, a:e],
                        start=(b == 0),
                        stop=(b == njp - 1),
                    )
                if b < njs:
                    sm = wpool.tile([128, C], f32r, tag="sm")
                    nc.vector.tensor_scalar(
                        out=sm[:], in0=d2[:], scalar1=SEP2, scalar2=None,
                        op0=ALU.is_le
                    )
                    for a, e in chunks:
                        nc.tensor.matmul(
                            out=accs[:, a:e],
                            lhsT=jw[:, 9:14],
                            rhs=sm[:, a:e],
                            start=(b == 0),
                            stop=(b == njs - 1),
                        )

            po = opool.tile([9, C], f32)
            nc.scalar.copy(out=po[:], in_=accp[:])
            so = opool.tile([5, C], f32)
            nc.scalar.copy(out=so[:], in_=accs[:])
            nc.sync.dma_start(out=outp_h[:, :], in_=po[:])
            nc.sync.dma_start(out=outs_h[:, :], in_=so[:])
    nc.finalize()
    return nc


def _get_nc(C: int, njp: int, njs: int) -> bass.Bass:
    key = (C, njp, njs)
    if key not in _CACHE:
        _CACHE[key] = _build(C, njp, njs)
    return _CACHE[key]


def _hilo(v64):
    hi = np.round(v64 * 1024.0) / 1024.0
    lo = (v64 - hi).astype(np.float32)
    return hi.astype(np.float32), lo


def _prepare(pos, vel):
    """Host-side prep: patch sort, lattice-exact shifts, block culling,
    packed device inputs.

    Returns (in_maps, sels, qxcs, qycs, C, njp, njs).
    """
    n = pos.shape[0]
    assert n == N, f"expected {N} boids, got {n}"

    # --- patch assignment: 4 columns in x (0.25), 2 rows in y (0.5) ---
    ix = np.clip((pos[:, 0] * 4.0).astype(np.int64), 0, 3)
    iy = np.clip((pos[:, 1] * 2.0).astype(np.int64), 0, 1)
    patch = ix * 2 + iy
    counts = np.bincount(patch, minlength=NCORES)
    C = int(np.ceil(counts.max() / 64.0) * 64)
    C = max(C, 512)

    # --- candidate order: x-column (0.125) major, y minor -> tight blocks ---
    jcol = np.clip((pos[:, 0] * 8.0).astype(np.int64), 0, 7)
    order = np.lexsort((pos[:, 1], jcol))

    # --- lattice-exact coordinate shifts ---
    SCL = 1 << 23
    p64x = pos[:, 0].astype(np.float64)
    p64y = pos[:, 1].astype(np.float64)
    kx = np.round(p64x * SCL).astype(np.int64)
    ky = np.round(p64y * SCL).astype(np.int64)
    lattice = bool(
        np.all(kx.astype(np.float64) == p64x * SCL)
        and np.all(ky.astype(np.float64) == p64y * SCL)
        and kx.min() >= 0 and kx.max() < SCL
        and ky.min() >= 0 and ky.max() < SCL
    )

    vx64 = vel[:, 0].astype(np.float64)
    vy64 = vel[:, 1].astype(np.float64)

    percore = []
    for c in range(NCORES):
        sel = np.nonzero(patch == c)[0]
        axk = int(((c // 2) * 0.25 + 0.125) * SCL)
        ayk = int(((c % 2) * 0.5 + 0.25) * SCL)
        if lattice:
            sxk = (kx - axk + (SCL >> 1)) % SCL
            syk = (ky - ayk + (SCL >> 1)) % SCL
            cx = (sxk.astype(np.float64) / SCL).astype(np.float32)
            cy = (syk.astype(np.float64) / SCL).astype(np.float32)
        else:  # fallback: tiny (~1e-9) inexactness vs reference wrap
            cx = np.mod(p64x - axk / SCL + 0.5, 1.0).astype(np.float32)
            cy = np.mod(p64y - ayk / SCL + 0.5, 1.0).astype(np.float32)

        # distance from each (shifted, sorted) candidate to the patch rect
        scx = cx[order].astype(np.float64)
        scy = cy[order].astype(np.float64)
        ddx = np.maximum(np.abs(scx - 0.5) - HX, 0.0)
        ddy = np.maximum(np.abs(scy - 0.5) - HY, 0.0)
        dd2 = ddx * ddx + ddy * ddy
        bd2 = dd2.reshape(NBLK, 128).min(axis=1)
        bsep = np.nonzero(bd2 <= RCULL_S * RCULL_S)[0]
        bperc = np.nonzero((bd2 <= RCULL_P * RCULL_P) & (bd2 > RCULL_S * RCULL_S))[0]
        percore.append((sel, cx, cy, scx, scy, bsep, bperc))

    njs = max(len(pc[5]) for pc in percore)
    njp = njs + max(len(pc[6]) for pc in percore)

    in_maps = []
    sels = []
    qxcs = []
    qycs = []
    for c in range(NCORES):
        sel, cx, cy, scx, scy, bsep, bperc = percore[c]
        nq = len(sel)
        qxp = np.full(C, 0.5, np.float32)
        qxp[:nq] = cx[sel]
        qyp = np.full(C, 0.5, np.float32)
        qyp[:nq] = cy[sel]
        qxb = np.ascontiguousarray(np.broadcast_to(qxp, (128, C)))
        qyb = np.ascontiguousarray(np.broadcast_to(qyp, (128, C)))

        # block list: sep-active first (padded to njs), then perc-only
        blocks = list(bsep) + [-1] * (njs - len(bsep)) + list(bperc)
        blocks += [-1] * (njp - len(blocks))

        scx32 = scx.astype(np.float32)
        scy32 = scy.astype(np.float32)
        svx = vx64[order]
        svy = vy64[order]
        jd = np.empty((njp, 128, 16), np.float32)
        for bi, b in enumerate(blocks):
            if b < 0:  # sentinel: far away, zero contributions
                jd[bi] = 0.0
                jd[bi, :, 0] = -50.0
                jd[bi, :, 1] = -50.0
                continue
            s = slice(128 * b, 128 * (b + 1))
            pxhi, pxlo = _hilo(scx[s] - 0.5)
            pyhi, pylo = _hilo(scy[s] - 0.5)
            vxhi, vxlo = _hilo(svx[s])
            vyhi, vylo = _hilo(svy[s])
            jd[bi, :, 0] = -scx32[s]
            jd[bi, :, 1] = -scy32[s]
            jd[bi, :, 2] = 1.0
            jd[bi, :, 3] = vxhi
            jd[bi, :, 4] = vxlo
            jd[bi, :, 5] = vyhi
            jd[bi, :, 6] = vylo
            jd[bi, :, 7] = pxhi
            jd[bi, :, 8] = pxlo
            jd[bi, :, 9] = pyhi
            jd[bi, :, 10] = pylo
            jd[bi, :, 11] = 1.0
            jd[bi, :, 12] = pxhi
            jd[bi, :, 13] = pxlo
            jd[bi, :, 14] = pyhi
            jd[bi, :, 15] = pylo

        in_maps.append({"qxb": qxb, "qyb": qyb, "jd": jd})
        sels.append(sel)
        qxcs.append(qxp[:nq].astype(np.float64) - 0.5)
        qycs.append(qyp[:nq].astype(np.float64) - 0.5)
    return in_maps, sels, qxcs, qycs, C, njp, njs


def kernel(position, velocity, noise, separation_weight, alignment_weight,
           cohesion_weight, noise_scale):
    pos = np.asarray(position, dtype=np.float32)
    vel = np.asarray(velocity, dtype=np.float32)
    noi = np.asarray(noise, dtype=np.float32)
    ws = float(separation_weight)
    wa = float(alignment_weight)
    wc = float(cohesion_weight)
    nsc = float(noise_scale)

    in_maps, sels, qxcs, qycs, C, njp, njs = _prepare(pos, vel)
    vx64 = vel[:, 0].astype(np.float64)
    vy64 = vel[:, 1].astype(np.float64)

    nc = _get_nc(C, njp, njs)
    res = run_bass_kernel_spmd(nc, in_maps, list(range(NCORES))).results

    # --- host epilogue (f64) ---
    out = np.zeros((N, 2), np.float32)
    for c in range(NCORES):
        sel = sels[c]
        nq = len(sel)
        P = res[c]["outp"].astype(np.float64)
        S = res[c]["outs"].astype(np.float64)
        cnt_all = P[0, :nq]
        svx, svy = P[1, :nq] + P[2, :nq], P[3, :nq] + P[4, :nq]
        spx, spy = P[5, :nq] + P[6, :nq], P[7, :nq] + P[8, :nq]
        scn = S[0, :nq]
        ssx, ssy = S[1, :nq] + S[2, :nq], S[3, :nq] + S[4, :nq]
        qxc, qyc = qxcs[c], qycs[c]

        cnt = cnt_all - 1.0
        # alignment: mean neighbor velocity minus own velocity
        vax = (svx - vx64[sel]) / cnt
        vay = (svy - vy64[sel]) / cnt
        dvx = vax - vx64[sel]
        dvy = vay - vy64[sel]
        # cohesion: mean toroidal diff (self term cancels exactly)
        pax = (spx - qxc * cnt_all) / cnt
        pay = (spy - qyc * cnt_all) / cnt
        # separation: -sum of masked diffs
        sepx = -(ssx - qxc * scn)
        sepy = -(ssy - qyc * scn)

        n1 = np.maximum(np.sqrt(sepx * sepx + sepy * sepy), EPS)
        n2 = np.maximum(np.sqrt(dvx * dvx + dvy * dvy), EPS)
        n3 = np.maximum(np.sqrt(pax * pax + pay * pay), EPS)

        ax = ws * sepx / n1 + wa * dvx / n2 + wc * pax / n3
        ay = ws * sepy / n1 + wa * dvy / n2 + wc * pay / n3
        ax = ax + nsc * noi[sel, 0].astype(np.float64)
        ay = ay + nsc * noi[sel, 1].astype(np.float64)
        nn = np.sqrt(ax * ax + ay * ay)
        f = np.where(nn > 1.0, 1.0 / np.maximum(nn, EPS), 1.0)
        out[sel, 0] = (ax * f).astype(np.float32)
        out[sel, 1] = (ay * f).astype(np.float32)
    return out


def run_with_trace(np_inputs):
    """Debug helper for test.py: run the device program with trace=True and
    return (exec_time_ns, profile_json_path_or_None)."""
    pos = np.asarray(np_inputs["position"], dtype=np.float32)
    vel = np.asarray(np_inputs["velocity"], dtype=np.float32)
    in_maps, _, _, _, C, njp, njs = _prepare(pos, vel)
    nc = _get_nc(C, njp, njs)
    r = run_bass_kernel_spmd(nc, in_maps, list(range(NCORES)), trace=True)
    return getattr(r, "exec_time_ns", None), getattr(r, "profile_json", None)


# revision 27
# speedup vs baseline: 3.9419x; 1.1935x over previous
"""Boid policy kernel for Trainium2 (8 NeuronCores).

Strategy
--------
Host: sort boids into 8 spatial patches (4x2 grid of 0.25 x 0.5 cells).
Core c owns the queries of patch c. All coordinates handed to core c are
shifted (mod 1) so patch c is centered at (0.5, 0.5); positions produced by
jax.random.uniform are multiples of 2^-23, so the shift is bit-exact when
done in integer lattice space. With the patch centered, every candidate pair
within unwrapped distance 0.5 has its unwrapped diff equal to the exact
toroidal diff, and every pair beyond is outside both interaction radii
either way -> no per-pair wrap handling on device.

Candidates are additionally sorted by (x-column of 0.125, then y) so each
128-candidate block is a tight spatial cluster. The host culls blocks per
core: blocks beyond perception reach of the patch rectangle are dropped
entirely; blocks beyond separation reach skip the separation mask/matmuls.
Sep-active blocks are ordered first so the device program is static.

Device (per core, j = candidate on partitions, i = query on free axis):
  dx2 = ACT Square(qx_bcast + (-cx_j))      (per-partition bias)
  dy2 = ACT Square(qy_bcast + (-cy_j))
  d2  = GPSIMD tensor_tensor add            (exact fp32, frees the DVE)
  perc = DVE (d2 <= 0.2^2) [f32r out], sep likewise on sep-active blocks
  PE: masked sums via matmuls with the tiny jdata matrix as stationary
      operand and the mask tile as moving operand (float32r), accumulated
      over the active blocks in PSUM:
        perc-sums: [count, vxhi, vxlo, vyhi, vylo, pxhi, pxlo, pyhi, pylo]
        sep-sums:  [count, pxhi, pxlo, pyhi, pylo]
      (hi/lo weight splits survive the f32r tfloat32 rounding exactly)

Host epilogue (f64): recover sum(mask*diff) = sum(mask*pc) - qc*count
(self-pair cancels), subtract self from count/velocity sums, normalize the
three steers, combine with weights, add noise, clip by norm.
"""

import numpy as np

import concourse.bass as bass
import concourse.bacc as bacc
import concourse.mybir as mybir
from concourse.tile import TileContext
from concourse.bass_utils import run_bass_kernel_spmd

N = 8192
NCORES = 8
NBLK = N // 128  # 64 candidate blocks
PERC2 = float(np.float32(0.2**2))
SEP2 = float(np.float32(0.02**2))
EPS = 1e-8
HX, HY = 0.125, 0.25  # patch half-extents
RCULL_P = 0.2 + 1e-3
RCULL_S = 0.02 + 1e-3

_CACHE = {}


def _build(C: int, njp: int, njs: int) -> bass.Bass:
    f32 = mybir.dt.float32
    f32r = mybir.dt.float32r
    AF = mybir.ActivationFunctionType
    ALU = mybir.AluOpType

    nc = bacc.Bacc()
    qxb_h = nc.declare_dram_parameter("qxb", [128, C], f32, isOutput=False)
    qyb_h = nc.declare_dram_parameter("qyb", [128, C], f32, isOutput=False)
    # jd cols: [0]=-cx [1]=-cy [2:11]=perc weights [1,vxhi,vxlo,vyhi,vylo,
    # pxhi,pxlo,pyhi,pylo] [11:16]=sep weights [1,pxhi,pxlo,pyhi,pylo]
    jd_h = nc.declare_dram_parameter("jd", [njp, 128, 16], f32, isOutput=False)
    outp_h = nc.declare_dram_parameter("outp", [9, C], f32, isOutput=True)
    outs_h = nc.declare_dram_parameter("outs", [5, C], f32, isOutput=True)

    chunks = [(s, min(s + 512, C)) for s in range(0, C, 512)]
    CSPL = (int(C * 0.55) + 63) & ~63  # DVE share of the d2 add; rest on GPSIMD

    with TileContext(nc) as tc:
        with (
            tc.tile_pool(name="const", bufs=1) as cpool,
            tc.tile_pool(name="jin", bufs=4) as jpool,
            tc.tile_pool(name="work", bufs=4) as wpool,
            tc.tile_pool(name="outb", bufs=1) as opool,
            tc.tile_pool(name="acc", bufs=1, space="PSUM") as apool,
        ):
            qx = cpool.tile([128, C], f32)
            nc.gpsimd.dma_start(out=qx[:], in_=qxb_h[:, :])
            qy = cpool.tile([128, C], f32)
            nc.gpsimd.dma_start(out=qy[:], in_=qyb_h[:, :])

            accp = apool.tile([9, C], f32)
            accs = apool.tile([5, C], f32)

            for b in range(njp):
                jt = jpool.tile([128, 16], f32)
                nc.sync.dma_start(out=jt[:], in_=jd_h[b])
                # weights copy, rounded to f32r on write (hi cols exact,
                # lo cols' rounding bounded by 2^-21)
                jw = jpool.tile([128, 14], f32r, tag="jw")
                nc.vector.tensor_copy(out=jw[:], in_=jt[:, 2:16])

                dx2 = wpool.tile([128, C], f32, tag="dx2")
                nc.scalar.activation(
                    out=dx2[:], in_=qx[:], func=AF.Square, bias=jt[:, 0:1], scale=1.0
                )
                dy2 = wpool.tile([128, C], f32, tag="dy2")
                nc.scalar.activation(
                    out=dy2[:], in_=qy[:], func=AF.Square, bias=jt[:, 1:2], scale=1.0
                )
                # d2 add split by column range: DVE and GPSIMD in parallel
                d2 = wpool.tile([128, C], f32, tag="d2")
                nc.vector.tensor_tensor(
                    out=d2[:, 0:CSPL], in0=dx2[:, 0:CSPL], in1=dy2[:, 0:CSPL],
                    op=ALU.add
                )
                nc.gpsimd.tensor_tensor(
                    out=d2[:, CSPL:C], in0=dx2[:, CSPL:C], in1=dy2[:, CSPL:C],
                    op=ALU.add
                )
                pm = wpool.tile([128, C], f32r, tag="pm")
                nc.vector.tensor_scalar(
                    out=pm[:], in0=d2[:], scalar1=PERC2, scalar2=None, op0=ALU.is_le
                )
                for a, e in chunks:
                    nc.tensor.matmul(
                        out=accp[:, a:e],
                        lhsT=jw[:, 0:9],
                        rhs=pm[:, a:e],
                        start=(b == 0),
                        stop=(b == njp - 1),
                    )
                if b < njs:
                    sm = wpool.tile([128, C], f32r, tag="sm")
                    nc.vector.tensor_scalar(
                        out=sm[:], in0=d2[:], scalar1=SEP2, scalar2=None,
                        op0=ALU.is_le
                    )
                    for a, e in chunks:
                        nc.tensor.matmul(
                            out=accs[:, a:e],
                            lhsT=jw[:, 9:14],
                            rhs=sm[:, a:e],
                            start=(b == 0),
                            stop=(b == njs - 1),
                        )

            po = opool.tile([9, C], f32)
            nc.scalar.copy(out=po[:], in_=accp[:])
            so = opool.tile([5, C], f32)
            nc.scalar.copy(out=so[:], in_=accs[:])
            nc.sync.dma_start(out=outp_h[:, :], in_=po[:])
            nc.sync.dma_start(out=outs_h[:, :], in_=so[:])
    nc.finalize()
    return nc


def _get_nc(C: int, njp: int, njs: int) -> bass.Bass:
    key = (C, njp, njs)
    if key not in _CACHE:
        _CACHE[key] = _build(C, njp, njs)
    return _CACHE[key]


def _hilo(v64):
    hi = np.round(v64 * 1024.0) / 1024.0
    lo = (v64 - hi).astype(np.float32)
    return hi.astype(np.float32), lo


def _prepare(pos, vel):
    """Host-side prep: patch sort, lattice-exact shifts, block culling,
    packed device inputs.

    Returns (in_maps, sels, qxcs, qycs, C, njp, njs).
    """
    n = pos.shape[0]
    assert n == N, f"expected {N} boids, got {n}"

    # --- patch assignment: 4 columns in x (0.25), 2 rows in y (0.5) ---
    ix = np.clip((pos[:, 0] * 4.0).astype(np.int64), 0, 3)
    iy = np.clip((pos[:, 1] * 2.0).astype(np.int64), 0, 1)
    patch = ix * 2 + iy
    counts = np.bincount(patch, minlength=NCORES)
    C = int(np.ceil(counts.max() / 64.0) * 64)
    C = max(C, 512)

    # --- candidate order: x-column (0.125) major, y minor -> tight blocks ---
    jcol = np.clip((pos[:, 0] * 8.0).astype(np.int64), 0, 7)
    order = np.lexsort((pos[:, 1], jcol))

    # --- lattice-exact coordinate shifts ---
    SCL = 1 << 23
    p64x = pos[:, 0].astype(np.float64)
    p64y = pos[:, 1].astype(np.float64)
    kx = np.round(p64x * SCL).astype(np.int64)
    ky = np.round(p64y * SCL).astype(np.int64)
    lattice = bool(
        np.all(kx.astype(np.float64) == p64x * SCL)
        and np.all(ky.astype(np.float64) == p64y * SCL)
        and kx.min() >= 0 and kx.max() < SCL
        and ky.min() >= 0 and ky.max() < SCL
    )

    vx64 = vel[:, 0].astype(np.float64)
    vy64 = vel[:, 1].astype(np.float64)

    percore = []
    for c in range(NCORES):
        sel = np.nonzero(patch == c)[0]
        axk = int(((c // 2) * 0.25 + 0.125) * SCL)
        ayk = int(((c % 2) * 0.5 + 0.25) * SCL)
        if lattice:
            sxk = (kx - axk + (SCL >> 1)) % SCL
            syk = (ky - ayk + (SCL >> 1)) % SCL
            cx = (sxk.astype(np.float64) / SCL).astype(np.float32)
            cy = (syk.astype(np.float64) / SCL).astype(np.float32)
        else:  # fallback: tiny (~1e-9) inexactness vs reference wrap
            cx = np.mod(p64x - axk / SCL + 0.5, 1.0).astype(np.float32)
            cy = np.mod(p64y - ayk / SCL + 0.5, 1.0).astype(np.float32)

        # distance from each (shifted, sorted) candidate to the patch rect
        scx = cx[order].astype(np.float64)
        scy = cy[order].astype(np.float64)
        ddx = np.maximum(np.abs(scx - 0.5) - HX, 0.0)
        ddy = np.maximum(np.abs(scy - 0.5) - HY, 0.0)
        dd2 = ddx * ddx + ddy * ddy
        bd2 = dd2.reshape(NBLK, 128).min(axis=1)
        bsep = np.nonzero(bd2 <= RCULL_S * RCULL_S)[0]
        bperc = np.nonzero((bd2 <= RCULL_P * RCULL_P) & (bd2 > RCULL_S * RCULL_S))[0]
        percore.append((sel, cx, cy, scx, scy, bsep, bperc))

    njs = max(len(pc[5]) for pc in percore)
    njp = njs + max(len(pc[6]) for pc in percore)

    in_maps = []
    sels = []
    qxcs = []
    qycs = []
    for c in range(NCORES):
        sel, cx, cy, scx, scy, bsep, bperc = percore[c]
        nq = len(sel)
        qxp = np.full(C, 0.5, np.float32)
        qxp[:nq] = cx[sel]
        qyp = np.full(C, 0.5, np.float32)
        qyp[:nq] = cy[sel]
        qxb = np.ascontiguousarray(np.broadcast_to(qxp, (128, C)))
        qyb = np.ascontiguousarray(np.broadcast_to(qyp, (128, C)))

        # block list: sep-active first (padded to njs), then perc-only
        blocks = list(bsep) + [-1] * (njs - len(bsep)) + list(bperc)
        blocks += [-1] * (njp - len(blocks))

        scx32 = scx.astype(np.float32)
        scy32 = scy.astype(np.float32)
        svx = vx64[order]
        svy = vy64[order]
        jd = np.empty((njp, 128, 16), np.float32)
        for bi, b in enumerate(blocks):
            if b < 0:  # sentinel: far away, zero contributions
                jd[bi] = 0.0
                jd[bi, :, 0] = -50.0
                jd[bi, :, 1] = -50.0
                continue
            s = slice(128 * b, 128 * (b + 1))
            pxhi, pxlo = _hilo(scx[s] - 0.5)
            pyhi, pylo = _hilo(scy[s] - 0.5)
            vxhi, vxlo = _hilo(svx[s])
            vyhi, vylo = _hilo(svy[s])
            jd[bi, :, 0] = -scx32[s]
            jd[bi, :, 1] = -scy32[s]
            jd[bi, :, 2] = 1.0
            jd[bi, :, 3] = vxhi
            jd[bi, :, 4] = vxlo
            jd[bi, :, 5] = vyhi
            jd[bi, :, 6] = vylo
            jd[bi, :, 7] = pxhi
            jd[bi, :, 8] = pxlo
            jd[bi, :, 9] = pyhi
            jd[bi, :, 10] = pylo
            jd[bi, :, 11] = 1.0
            jd[bi, :, 12] = pxhi
            jd[bi, :, 13] = pxlo
            jd[bi, :, 14] = pyhi
            jd[bi, :, 15] = pylo

        in_maps.append({"qxb": qxb, "qyb": qyb, "jd": jd})
        sels.append(sel)
        qxcs.append(qxp[:nq].astype(np.float64) - 0.5)
        qycs.append(qyp[:nq].astype(np.float64) - 0.5)
    return in_maps, sels, qxcs, qycs, C, njp, njs


def kernel(position, velocity, noise, separation_weight, alignment_weight,
           cohesion_weight, noise_scale):
    pos = np.asarray(position, dtype=np.float32)
    vel = np.asarray(velocity, dtype=np.float32)
    noi = np.asarray(noise, dtype=np.float32)
    ws = float(separation_weight)
    wa = float(alignment_weight)
    wc = float(cohesion_weight)
    nsc = float(noise_scale)

    in_maps, sels, qxcs, qycs, C, njp, njs = _prepare(pos, vel)
    vx64 = vel[:, 0].astype(np.float64)
    vy64 = vel[:, 1].astype(np.float64)

    nc = _get_nc(C, njp, njs)
    res = run_bass_kernel_spmd(nc, in_maps, list(range(NCORES))).results

    # --- host epilogue (f64) ---
    out = np.zeros((N, 2), np.float32)
    for c in range(NCORES):
        sel = sels[c]
        nq = len(sel)
        P = res[c]["outp"].astype(np.float64)
        S = res[c]["outs"].astype(np.float64)
        cnt_all = P[0, :nq]
        svx, svy = P[1, :nq] + P[2, :nq], P[3, :nq] + P[4, :nq]
        spx, spy = P[5, :nq] + P[6, :nq], P[7, :nq] + P[8, :nq]
        scn = S[0, :nq]
        ssx, ssy = S[1, :nq] + S[2, :nq], S[3, :nq] + S[4, :nq]
        qxc, qyc = qxcs[c], qycs[c]

        cnt = cnt_all - 1.0
        # alignment: mean neighbor velocity minus own velocity
        vax = (svx - vx64[sel]) / cnt
        vay = (svy - vy64[sel]) / cnt
        dvx = vax - vx64[sel]
        dvy = vay - vy64[sel]
        # cohesion: mean toroidal diff (self term cancels exactly)
        pax = (spx - qxc * cnt_all) / cnt
        pay = (spy - qyc * cnt_all) / cnt
        # separation: -sum of masked diffs
        sepx = -(ssx - qxc * scn)
        sepy = -(ssy - qyc * scn)

        n1 = np.maximum(np.sqrt(sepx * sepx + sepy * sepy), EPS)
        n2 = np.maximum(np.sqrt(dvx * dvx + dvy * dvy), EPS)
        n3 = np.maximum(np.sqrt(pax * pax + pay * pay), EPS)

        ax = ws * sepx / n1 + wa * dvx / n2 + wc * pax / n3
        ay = ws * sepy / n1 + wa * dvy / n2 + wc * pay / n3
        ax = ax + nsc * noi[sel, 0].astype(np.float64)
        ay = ay + nsc * noi[sel, 1].astype(np.float64)
        nn = np.sqrt(ax * ax + ay * ay)
        f = np.where(nn > 1.0, 1.0 / np.maximum(nn, EPS), 1.0)
        out[sel, 0] = (ax * f).astype(np.float32)
        out[sel, 1] = (ay * f).astype(np.float32)
    return out


def run_with_trace(np_inputs):
    """Debug helper for test.py: run the device program with trace=True and
    return (exec_time_ns, profile_json_path_or_None)."""
    pos = np.asarray(np_inputs["position"], dtype=np.float32)
    vel = np.asarray(np_inputs["velocity"], dtype=np.float32)
    in_maps, _, _, _, C, njp, njs = _prepare(pos, vel)
    nc = _get_nc(C, njp, njs)
    r = run_bass_kernel_spmd(nc, in_maps, list(range(NCORES)), trace=True)
    return getattr(r, "exec_time_ns", None), getattr(r, "profile_json", None)
